# revision 31
# baseline (speedup 1.0000x reference)
"""MoE layer (top-2 of 8 experts, SwiGLU) on 8 Trainium2 NeuronCores.

Strategy: expert parallelism with capacity-limited dispatch. The router
(softmax top-2 over a [8192, 8] logit matrix) runs on the host in numpy;
tokens are gathered per expert up to a capacity of C = min(2048, padded
max count) and each core runs one expert's three GEMMs with its weights
resident in SBUF (bf16 operands, fp32 PSUM accumulation). The rare
overflow tokens beyond capacity (~100 of 16384 token-expert pairs) are
computed on the host in fp32 during the device call's result scatter.
The host applies the gate weights during the scatter-add back to the
full output, so padded rows simply contribute nothing.

Device layout is feature-on-partition / token-on-free:
  G.T = Wg.T.T @ X.T   per (H-chunk, D-chunk) tile, accumulated over D
  U.T = W1.T.T @ X.T
  h   = silu(G) * U    (ScalarE silu, VectorE multiply, bf16 result)
  Y.T = W2.T.T @ h     accumulated over H-chunks

H = 2736 = 21*128 + 48: the ragged 48-row tail of the Wg and W1 GEMMs is
merged into a single stationary [128, 112] (wg tail at psum partitions
0:48, w1 tail at 64:112 -- partition access must be 32-aligned), saving 8
matmuls per token tile. W2's tail chunk contracts over 48 partitions only.

Each dma_start trigger costs ~650ns serially on its HWDGE engine, so all
operands are laid out host-side in device tile order ([128, cols] blocks)
and move with ONE trigger per weight group / token tile. Every DMA rides
the single Sync (SP) HWDGE queue: splitting loads across the two HWDGE
queues was observed to corrupt results intermittently (the Tile DMA
semaphore accounting assumes FIFO completion, which two independently
draining rings violate). Program order keeps all input triggers ahead of
output triggers, so the serial queue costs nothing.
"""

import numpy as np
import ml_dtypes

B, S, D = 2, 4096, 1024
E, H, TOPK = 8, 2736, 2
T = B * S
KD = D // 128  # 8 contraction chunks over D
KHF = H // 128  # 21 full 128-row H chunks
HF = KHF * 128  # 2688
TAIL = H - HF  # 48
KH = KHF + 1  # 22 chunks including the ragged tail
N_CORES = 8
CT = 512  # max token tile (free dim per matmul, one PSUM bank of fp32)
CAP = 2048  # expert capacity (= mean load T*TOPK/E); overflow -> host
GRPS = [2, 2, 4, 6, 7]  # wg/w1 H-chunks per load group, finest first
W2GRPS = [8, 7, 7]  # w2 H-chunks per load group (incl zero-padded tail)

_BF16 = ml_dtypes.bfloat16


def _install_drain_patch():
    """walrus in this image rejects any instruction carrying >1 sync wait
    ("Too many sync wait commands"). Split waits: every instruction keeps one
    wait; extra waits ride dedicated NoOps inserted just before it on the
    same engine. Applies to the Tile-lowered stream and to the tail drain."""
    import concourse.mybir as mybir
    import concourse.tile as tile
    from concourse.vector_clock import ScopedClock

    if getattr(tile.TileContext, "_drain_patch_installed", False):
        return

    _orig_lower = tile.TileContext._lower_ordered_insts

    def _split_lower(self, ordered):
        nc = self.nc
        for bb_name, insts in ordered.items():
            new = []
            for inst in insts:
                si = inst.sync_info
                ow = list(si.on_wait) if si is not None and si.on_wait else []
                if len(ow) > 1:
                    scopes = self._inst_to_scopes.get(inst.name, ())
                    for w in ow[:-1]:
                        nop = mybir.InstNoOp(
                            name=nc.get_next_instruction_name(),
                            engine=inst.engine,
                            ins=[],
                            outs=[],
                            sync_info=mybir.SyncInfo(on_wait=[w], on_update=[]),
                            bass_nofuse=True,
                        )
                        if scopes:
                            self._inst_to_scopes[nop.name] = scopes
                        new.append(nop)
                    ou = list(si.on_update) if si.on_update else []
                    inst.sync_info = mybir.SyncInfo(on_wait=[ow[-1]], on_update=ou)
                new.append(inst)
            ordered[bb_name] = new
        return _orig_lower(self, ordered)

    tile.TileContext._lower_ordered_insts = _split_lower

    def _patched(self, tick_clock, wait_clock):
        nc = self.nc
        nops = [nc.sync.nop(nofuse=True) for _ in range(30)]
        drain_inst = nc.sync.drain()
        wait_clock.add_sem_waits(
            drain_inst.ins, ScopedClock({None: tick_clock.global_clock})
        )
        si = drain_inst.ins.sync_info
        ow = list(si.on_wait) if si is not None and si.on_wait else []
        if len(ow) > 1:
            assert len(ow) <= 1 + len(nops), f"drain needs {len(ow)} waits"
            for i, w in enumerate(ow[:-1]):
                nops[i].ins.sync_info = mybir.SyncInfo(on_wait=[w], on_update=[])
            ou = list(si.on_update) if si.on_update else []
            drain_inst.ins.sync_info = mybir.SyncInfo(on_wait=[ow[-1]], on_update=ou)
        nc.all_engine_barrier()
        assert self.sems is not None
        popped = nc._tile_sem_poison_stack.pop()
        assert popped is self._sem_poison
        nc.clear_and_free_semaphores(list(self.sems.allocated().values()))
        nc.all_engine_barrier()

    tile.TileContext._drain_and_barrier = _patched
    tile.TileContext._drain_patch_installed = True


def _token_tiles(C):
    """Tile sizes (multiples of 32, each <=512) with the moving dim large
    enough to hide LDWEIGHTS. The first tile is ~C/8 so the ramp's upfront
    x-load DMA debt (which the PE must wait out) is halved; the rest are
    near-equal."""
    sizes = []
    rest = C
    if C >= 2 * CT:
        first = max(128, (C // 8) // 32 * 32)
        sizes.append(first)
        rest = C - first
    n = -(-rest // CT)
    base = (rest // n) // 32 * 32
    mids = [base] * n
    extra = (rest - base * n) // 32
    for i in range(extra):
        mids[i] += 32
    sizes += mids
    tiles = []
    t0 = 0
    for ct in sizes:
        tiles.append((t0, ct))
        t0 += ct
    assert t0 == C, (C, sizes)
    return tiles


_GRP_OFF = [0]
for _s in GRPS:
    _GRP_OFF.append(_GRP_OFF[-1] + _s)
_HK_MAP = []  # hk -> (group, offset within group)
for _g, _s in enumerate(GRPS):
    for _o in range(_s):
        _HK_MAP.append((_g, _o))

_PROGRAM_CACHE = {}


def _build_program(C):
    """One SPMD program: expert FFN over C (padded) tokens."""
    if C in _PROGRAM_CACHE:
        return _PROGRAM_CACHE[C]

    _install_drain_patch()
    import concourse.bass as bass
    import concourse.mybir as mybir
    import concourse.tile as tile

    bf16 = mybir.dt.bfloat16
    f32 = mybir.dt.float32

    nc = bass.Bass()
    # All operands are pre-arranged host-side into [128, cols] device tile
    # layout so each logical group moves with a single contiguous DMA.
    xD = nc.declare_dram_parameter("xD", [128, KD * C], bf16, isOutput=False)
    gwD = nc.declare_dram_parameter(
        "gwD", [128, 2 * KD * HF], bf16, isOutput=False
    )
    wtD = nc.declare_dram_parameter("wtD", [128, KD * 112], bf16, isOutput=False)
    # w2 includes the ragged tail as a 22nd chunk zero-padded to 128 rows,
    # so every y matmul keeps a full-row stationary (partial-row stationaries
    # break LDWEIGHTS pull-ahead and stall the PE ~200ns per chain).
    w2D = nc.declare_dram_parameter("w2D", [128, KH * D], bf16, isOutput=False)
    yD = nc.declare_dram_parameter("yD", [128, KD * C], f32, isOutput=True)

    with tile.TileContext(nc) as tc:
        with (
            tc.tile_pool(name="wpool", bufs=1) as wpool,
            tc.tile_pool(name="xpool", bufs=2) as xpool,
            tc.tile_pool(name="hpool", bufs=1) as hpool,
            tc.tile_pool(name="gpool", bufs=3) as gpool,
            tc.tile_pool(name="ypool", bufs=3) as ypool,
            tc.tile_pool(name="pg", bufs=2, space="PSUM") as pg,
            tc.tile_pool(name="pu", bufs=2, space="PSUM") as pu,
            tc.tile_pool(name="pt", bufs=1, space="PSUM") as pt,
            tc.tile_pool(name="py", bufs=3, space="PSUM") as py,
        ):
            tiles = _token_tiles(C)
            x_tiles = {}

            # PE warmup: the HAM clock gate holds the PE at half clock
            # until it sees ~3.4us of sustained activity. Run a chain of
            # throwaway matmuls on zeros (alternating pg/pu banks) while
            # the ramp DMAs are in flight, so the real stream starts at
            # full clock. The PE is otherwise idle here - this is free.
            ct_0 = tiles[0][1]
            warm = wpool.tile([128, ct_0], bf16, tag="warm")
            nc.vector.memset(warm[:], 0.0)
            # 10 matmuls bridge the gap from engine-ready (~8us) to first
            # real operands landing (~11.5us); once real matmuls flow the
            # PE stays busy and warms itself, so more would displace work.
            for i in range(10):
                if i % 2 == 0:
                    w_ps = pg.tile([128, ct_0], f32, tag="g_ps")
                else:
                    w_ps = pu.tile([128, ct_0], f32, tag="u_ps")
                nc.tensor.matmul(
                    w_ps[:], warm[:, 0:128], warm[:], start=True, stop=True
                )

            def load_x(t_idx, split=1):
                t0, ct = tiles[t_idx]
                x_s = xpool.tile([128, KD * ct], bf16, tag="x")
                w = KD * ct // split
                for c in range(split):
                    nc.sync.dma_start(
                        x_s[:, c * w:(c + 1) * w],
                        xD[:, KD * t0 + c * w:KD * t0 + (c + 1) * w],
                    )
                x_tiles[t_idx] = x_s

            # Ramp: tile 0's token slices load per-d, interleaved with the
            # finest wg/w1 group's quarters, so the first matmul chain
            # starts as early as possible (each dma_start trigger costs
            # ~650ns serially per engine). All input loads stay on the
            # Sync HWDGE queue: input loads on the Activation queue were
            # observed to race their consumers (intermittent corruption).
            x0 = xpool.tile([128, KD * ct_0], bf16, tag="x")
            x_tiles[0] = x0

            def load_x0_slice(d):
                nc.sync.dma_start(
                    x0[:, d * ct_0:(d + 1) * ct_0],
                    xD[:, d * ct_0:(d + 1) * ct_0],
                )

            n_grp = len(GRPS)
            gw_tiles = []
            for g in range(n_grp):
                cw = GRPS[g] * 128
                t = wpool.tile([128, 2 * KD * cw], bf16, tag=f"gw{g}")
                gw_tiles.append(t)
            cw0 = GRPS[0] * 128
            quart = KD * cw0 // 2
            for q in range(4):
                load_x0_slice(2 * q)
                load_x0_slice(2 * q + 1)
                nc.sync.dma_start(
                    gw_tiles[0][:, q * quart:(q + 1) * quart],
                    gwD[:, q * quart:(q + 1) * quart],
                )
            for g in range(1, n_grp):
                cw = GRPS[g] * 128
                base = 2 * KD * _GRP_OFF[g] * 128
                nc.sync.dma_start(
                    gw_tiles[g][:], gwD[:, base:base + 2 * KD * cw]
                )

            def wg_slice(d, hk):
                g, o = _HK_MAP[hk]
                cw = GRPS[g] * 128
                return gw_tiles[g][:, d * cw + o * 128:d * cw + (o + 1) * 128]

            def w1_slice(d, hk):
                g, o = _HK_MAP[hk]
                cw = GRPS[g] * 128
                base = KD * cw + d * cw + o * 128
                return gw_tiles[g][:, base:base + 128]

            # Merged ragged tail of wg/w1: [128, 112] per d-chunk.
            wt_s = wpool.tile([128, KD * 112], bf16, tag="wt")
            nc.sync.dma_start(wt_s[:], wtD[:])

            w2_tiles = []
            w2_off = [0]
            for s in W2GRPS:
                w2_off.append(w2_off[-1] + s)
            for g, s in enumerate(W2GRPS):
                t = wpool.tile([128, s * D], bf16, tag=f"w2{g}")
                nc.sync.dma_start(
                    t[:], w2D[:, w2_off[g] * D:w2_off[g + 1] * D]
                )
                w2_tiles.append(t)

            def w2_slice(hk, d):
                g = 0
                while hk >= w2_off[g + 1]:
                    g += 1
                base = (hk - w2_off[g]) * D + d * 128
                return w2_tiles[g][:, base:base + 128]

            for ti, (t0, ct) in enumerate(tiles):
                if ti + 1 < len(tiles):
                    load_x(ti + 1)
                x_s = x_tiles.pop(ti)

                h_s = hpool.tile([128, KH * ct], bf16, tag="h")
                for hk in range(KHF):
                    g_ps = pg.tile([128, ct], f32, tag="g_ps")
                    u_ps = pu.tile([128, ct], f32)
                    for d in range(KD):
                        nc.tensor.matmul(
                            g_ps[:],
                            wg_slice(d, hk),
                            x_s[:, d * ct:(d + 1) * ct],
                            start=(d == 0),
                            stop=(d == KD - 1),
                        )
                        nc.tensor.matmul(
                            u_ps[:],
                            w1_slice(d, hk),
                            x_s[:, d * ct:(d + 1) * ct],
                            start=(d == 0),
                            stop=(d == KD - 1),
                        )
                    g_tmp = gpool.tile([128, ct], f32, tag="g")
                    nc.scalar.activation(
                        g_tmp[:], g_ps[:], mybir.ActivationFunctionType.Silu
                    )
                    nc.vector.tensor_mul(
                        h_s[:, hk * ct:(hk + 1) * ct], g_tmp[:], u_ps[:]
                    )
                # ragged tail: one 8-matmul chain computes both g and u
                # (g rows on psum partitions 0:48, u rows on 64:112)
                t_ps = pt.tile([112, ct], f32, tag="t_ps")
                for d in range(KD):
                    nc.tensor.matmul(
                        t_ps[:],
                        wt_s[:, d * 112:(d + 1) * 112],
                        x_s[:, d * ct:(d + 1) * ct],
                        start=(d == 0),
                        stop=(d == KD - 1),
                    )
                gt_tmp = gpool.tile([TAIL, ct], f32, tag="gt")
                nc.scalar.activation(
                    gt_tmp[:], t_ps[0:TAIL], mybir.ActivationFunctionType.Silu
                )
                # rows TAIL:128 of the tail h chunk must be zeros (not
                # garbage) since the padded w2 tail chunk contracts over
                # all 128 partitions; memset the whole chunk first (offset
                # memsets are capped at 32 partitions) and let the mul
                # overwrite rows 0:TAIL.
                nc.vector.memset(h_s[:, KHF * ct:KH * ct], 0.0)
                nc.vector.tensor_mul(
                    h_s[0:TAIL, KHF * ct:KH * ct], gt_tmp[:], t_ps[64:64 + TAIL]
                )

                last = ti == len(tiles) - 1
                for d in range(KD):
                    if last and d == KD - 1:
                        # very last output chunk: compute/store in pieces
                        # so the copy+DMA after the final matmul is small.
                        # Piece width stays >=128 tokens so per-matmul time
                        # still covers the ~53ns FWL LDWEIGHTS.
                        k = 4 if ct >= 512 else (2 if ct >= 256 else 1)
                        hw = ct // k
                        for c0 in range(0, ct, hw):
                            y_ps = py.tile([128, hw], f32, tag="y_ps")
                            for hk in range(KH):
                                nc.tensor.matmul(
                                    y_ps[:],
                                    w2_slice(hk, d),
                                    h_s[:, hk * ct + c0:hk * ct + c0 + hw],
                                    start=(hk == 0),
                                    stop=(hk == KH - 1),
                                )
                            y_sb = ypool.tile([128, hw], f32, tag="yl")
                            nc.vector.tensor_copy(y_sb[:], y_ps[:])
                            nc.sync.dma_start(
                                yD[:, KD * t0 + d * ct + c0:
                                   KD * t0 + d * ct + c0 + hw],
                                y_sb[:],
                            )
                        continue
                    y_ps = py.tile([128, ct], f32)
                    for hk in range(KH):
                        nc.tensor.matmul(
                            y_ps[:],
                            w2_slice(hk, d),
                            h_s[:, hk * ct:(hk + 1) * ct],
                            start=(hk == 0),
                            stop=(hk == KH - 1),
                        )
                    # the last tile stores per-d so the final copy+DMA
                    # tail after the last matmul is as short as possible
                    if last:
                        y_sb = ypool.tile([128, ct], f32, tag="yl")
                        nc.vector.tensor_copy(y_sb[:], y_ps[:])
                        nc.sync.dma_start(
                            yD[:, KD * t0 + d * ct:KD * t0 + (d + 1) * ct],
                            y_sb[:],
                        )
                    elif d % 2 == 0:
                        y_pair = ypool.tile([128, 2 * ct], f32, tag="y")
                        nc.vector.tensor_copy(y_pair[:, 0:ct], y_ps[:])
                    else:
                        nc.vector.tensor_copy(y_pair[:, ct:2 * ct], y_ps[:])
                        nc.sync.dma_start(
                            yD[:, KD * t0 + (d - 1) * ct:KD * t0 + (d + 1) * ct],
                            y_pair[:],
                        )

    _PROGRAM_CACHE[C] = nc
    return nc


def _route(xf, gate_w):
    """Host router matching the reference: fp32 logits/softmax, top-2."""
    logits = xf @ gate_w.T  # [T, E] fp32
    m = logits.max(axis=1, keepdims=True)
    p = np.exp(logits - m, dtype=np.float32)
    p /= p.sum(axis=1, keepdims=True)
    # softmax is monotonic in logits, so top-2 by probs == top-2 by logits
    top_i = np.argsort(-p, axis=1, kind="stable")[:, :TOPK]  # [T, 2]
    top_p = np.take_along_axis(p, top_i, axis=1)
    gate_weights = top_p / (top_p.sum(axis=1, keepdims=True) + np.float32(1e-8))
    return top_i, gate_weights.astype(np.float32)


def _tileize(mat_T, splits):
    """[rows*128k, cols] -> [128, k*cols] blocks in device tile order.

    mat_T is [n*128, cols]; returns [128, n*cols] where block i holds
    rows i*128:(i+1)*128. `splits` optionally regroups the column axis
    first: a list of (c0, cw) column ranges, each emitted contiguously.
    """
    n = mat_T.shape[0] // 128
    a = mat_T.reshape(n, 128, mat_T.shape[1])
    blocks = []
    for c0, cw in splits:
        blocks.append(a[:, :, c0:c0 + cw].transpose(1, 0, 2).reshape(128, -1))
    return np.concatenate(blocks, axis=1) if len(blocks) > 1 else blocks[0]


def kernel(x, gate_w, Wg, W1, W2):
    from concourse.bass_utils import run_bass_kernel_spmd

    x = np.asarray(x, dtype=np.float32)
    gate_w = np.asarray(gate_w, dtype=np.float32)
    Wg = np.asarray(Wg, dtype=np.float32)
    W1 = np.asarray(W1, dtype=np.float32)
    W2 = np.asarray(W2, dtype=np.float32)

    xf = x.reshape(-1, D)
    top_i, gate_weights = _route(xf, gate_w)

    idx = [None] * E
    wts = [None] * E
    for e in range(E):
        rows, slots = np.nonzero(top_i == e)
        idx[e] = rows
        wts[e] = gate_weights[rows, slots]
    counts = np.array([len(i) for i in idx])
    C = min(CAP, max(128, int(np.ceil(counts.max() / 32)) * 32))
    dcounts = np.minimum(counts, C)

    nc = _build_program(C)
    tiles = _token_tiles(C)

    xf_bf = xf.astype(_BF16)
    in_maps = []
    for e in range(E):
        xT_e = np.zeros((D, C), dtype=_BF16)
        xT_e[:, : dcounts[e]] = xf_bf[idx[e][:C]].T
        wg_bf = Wg[e].astype(_BF16)  # [H, D]
        w1_bf = W1[e].astype(_BF16)
        # wg/w1 grouped blocks: per group g, [wg d0..d7 | w1 d0..d7]
        wgT = np.ascontiguousarray(wg_bf[:HF, :].T)  # [D, HF]
        w1T = np.ascontiguousarray(w1_bf[:HF, :].T)
        gw_blocks = []
        for g, s in enumerate(GRPS):
            c0, cw = _GRP_OFF[g] * 128, s * 128
            gw_blocks.append(_tileize(wgT, [(c0, cw)]))
            gw_blocks.append(_tileize(w1T, [(c0, cw)]))
        gwD_e = np.concatenate(gw_blocks, axis=1)
        # merged ragged tail [128, KD*112]
        wtT = np.zeros((D, 112), dtype=_BF16)
        wtT[:, 0:TAIL] = wg_bf[HF:H, :].T
        wtT[:, 64:64 + TAIL] = w1_bf[HF:H, :].T
        wtD_e = _tileize(wtT, [(0, 112)])
        w2T_pad = np.zeros((KH * 128, D), dtype=_BF16)
        w2T_pad[:H] = W2[e].astype(_BF16).T  # [H, D], tail rows zero
        in_maps.append({
            "xD": _tileize(xT_e, [(t0, ct) for t0, ct in tiles]),
            "gwD": gwD_e,
            "wtD": wtD_e,
            "w2D": _tileize(w2T_pad, [(0, D)]),
        })

    res = run_bass_kernel_spmd(nc, in_maps, list(range(N_CORES)))

    out = np.zeros((T, D), dtype=np.float32)
    for e in range(E):
        yD_e = res.results[e]["yD"]  # [128, KD*C] fp32, tile-major blocks
        yT_e = np.empty((C, D), dtype=np.float32)  # [token, D]
        for t0, ct in tiles:
            blk = yD_e[:, KD * t0:KD * (t0 + ct)].reshape(128, KD, ct)
            yT_e[t0:t0 + ct] = blk.transpose(2, 1, 0).reshape(ct, D)
        dev = idx[e][:C]
        out[dev] += wts[e][:C, None] * yT_e[: dcounts[e]]
        if counts[e] > C:  # host fp32 overflow path (capacity-limited MoE)
            ovf = idx[e][C:]
            xo = xf[ovf]
            go = xo @ Wg[e].T
            go = go / (1.0 + np.exp(-go))
            uo = xo @ W1[e].T
            yo = (go * uo) @ W2[e].T
            out[ovf] += wts[e][C:, None] * yo
    return out.reshape(B, S, D)


# revision 32
# speedup vs baseline: 1.0224x; 1.0224x over previous
"""MoE layer (top-2 of 8 experts, SwiGLU) on 8 Trainium2 NeuronCores.

Strategy: expert parallelism with capacity-limited dispatch. The router
(softmax top-2 over a [8192, 8] logit matrix) runs on the host in numpy;
tokens are gathered per expert up to a capacity of C = min(2048, padded
max count) and each core runs one expert's three GEMMs with its weights
resident in SBUF (bf16 operands, fp32 PSUM accumulation). The rare
overflow tokens beyond capacity (~100 of 16384 token-expert pairs) are
computed on the host in fp32 during the device call's result scatter.
The host applies the gate weights during the scatter-add back to the
full output, so padded rows simply contribute nothing.

Device layout is feature-on-partition / token-on-free:
  G.T = Wg.T.T @ X.T   per (H-chunk, D-chunk) tile, accumulated over D
  U.T = W1.T.T @ X.T
  h   = silu(G) * U    (ScalarE silu, VectorE multiply, bf16 result)
  Y.T = W2.T.T @ h     accumulated over H-chunks

H = 2736 = 21*128 + 48: the ragged 48-row tail of the Wg and W1 GEMMs is
merged into a single stationary [128, 112] (wg tail at psum partitions
0:48, w1 tail at 64:112 -- partition access must be 32-aligned), saving 8
matmuls per token tile. W2's tail chunk contracts over 48 partitions only.

Each dma_start trigger costs ~650ns serially on its HWDGE engine, so all
operands are laid out host-side in device tile order ([128, cols] blocks)
and move with ONE trigger per weight group / token tile. Every DMA rides
the single Sync (SP) HWDGE queue: splitting loads across the two HWDGE
queues was observed to corrupt results intermittently (the Tile DMA
semaphore accounting assumes FIFO completion, which two independently
draining rings violate). Program order keeps all input triggers ahead of
output triggers, so the serial queue costs nothing.
"""

import numpy as np
import ml_dtypes

B, S, D = 2, 4096, 1024
E, H, TOPK = 8, 2736, 2
T = B * S
KD = D // 128  # 8 contraction chunks over D
KHF = H // 128  # 21 full 128-row H chunks
HF = KHF * 128  # 2688
TAIL = H - HF  # 48
KH = KHF + 1  # 22 chunks including the ragged tail
N_CORES = 8
CT = 512  # max token tile (free dim per matmul, one PSUM bank of fp32)
CAP = 2048  # expert capacity (= mean load T*TOPK/E); overflow -> host
GRPS = [2, 2, 4, 6, 7]  # wg/w1 H-chunks per load group, finest first
W2GRPS = [8, 7, 7]  # w2 H-chunks per load group (incl zero-padded tail)

_BF16 = ml_dtypes.bfloat16


def _install_drain_patch():
    """walrus in this image rejects any instruction carrying >1 sync wait
    ("Too many sync wait commands"). Split waits: every instruction keeps one
    wait; extra waits ride dedicated NoOps inserted just before it on the
    same engine. Applies to the Tile-lowered stream and to the tail drain."""
    import concourse.mybir as mybir
    import concourse.tile as tile
    from concourse.vector_clock import ScopedClock

    if getattr(tile.TileContext, "_drain_patch_installed", False):
        return

    _orig_lower = tile.TileContext._lower_ordered_insts

    def _split_lower(self, ordered):
        nc = self.nc
        for bb_name, insts in ordered.items():
            new = []
            for inst in insts:
                si = inst.sync_info
                ow = list(si.on_wait) if si is not None and si.on_wait else []
                if len(ow) > 1:
                    scopes = self._inst_to_scopes.get(inst.name, ())
                    for w in ow[:-1]:
                        nop = mybir.InstNoOp(
                            name=nc.get_next_instruction_name(),
                            engine=inst.engine,
                            ins=[],
                            outs=[],
                            sync_info=mybir.SyncInfo(on_wait=[w], on_update=[]),
                            bass_nofuse=True,
                        )
                        if scopes:
                            self._inst_to_scopes[nop.name] = scopes
                        new.append(nop)
                    ou = list(si.on_update) if si.on_update else []
                    inst.sync_info = mybir.SyncInfo(on_wait=[ow[-1]], on_update=ou)
                new.append(inst)
            ordered[bb_name] = new
        return _orig_lower(self, ordered)

    tile.TileContext._lower_ordered_insts = _split_lower

    def _patched(self, tick_clock, wait_clock):
        nc = self.nc
        nops = [nc.sync.nop(nofuse=True) for _ in range(30)]
        drain_inst = nc.sync.drain()
        wait_clock.add_sem_waits(
            drain_inst.ins, ScopedClock({None: tick_clock.global_clock})
        )
        si = drain_inst.ins.sync_info
        ow = list(si.on_wait) if si is not None and si.on_wait else []
        if len(ow) > 1:
            assert len(ow) <= 1 + len(nops), f"drain needs {len(ow)} waits"
            for i, w in enumerate(ow[:-1]):
                nops[i].ins.sync_info = mybir.SyncInfo(on_wait=[w], on_update=[])
            ou = list(si.on_update) if si.on_update else []
            drain_inst.ins.sync_info = mybir.SyncInfo(on_wait=[ow[-1]], on_update=ou)
        nc.all_engine_barrier()
        assert self.sems is not None
        popped = nc._tile_sem_poison_stack.pop()
        assert popped is self._sem_poison
        nc.clear_and_free_semaphores(list(self.sems.allocated().values()))
        nc.all_engine_barrier()

    tile.TileContext._drain_and_barrier = _patched
    tile.TileContext._drain_patch_installed = True


def _token_tiles(C):
    """Near-equal tile sizes (multiples of 32, each <=512) so the matmul
    moving dim stays large enough to hide LDWEIGHTS."""
    n = -(-C // CT)
    base = (C // n) // 32 * 32
    sizes = [base] * n
    extra = (C - base * n) // 32
    for i in range(extra):
        sizes[i] += 32
    tiles = []
    t0 = 0
    for ct in sizes:
        tiles.append((t0, ct))
        t0 += ct
    assert t0 == C, (C, sizes)
    return tiles


_GRP_OFF = [0]
for _s in GRPS:
    _GRP_OFF.append(_GRP_OFF[-1] + _s)
_HK_MAP = []  # hk -> (group, offset within group)
for _g, _s in enumerate(GRPS):
    for _o in range(_s):
        _HK_MAP.append((_g, _o))

_PROGRAM_CACHE = {}


def _build_program(C):
    """One SPMD program: expert FFN over C (padded) tokens."""
    if C in _PROGRAM_CACHE:
        return _PROGRAM_CACHE[C]

    _install_drain_patch()
    import concourse.bass as bass
    import concourse.mybir as mybir
    import concourse.tile as tile

    bf16 = mybir.dt.bfloat16
    f32 = mybir.dt.float32

    nc = bass.Bass()
    # All operands are pre-arranged host-side into [128, cols] device tile
    # layout so each logical group moves with a single contiguous DMA.
    xD = nc.declare_dram_parameter("xD", [128, KD * C], bf16, isOutput=False)
    gwD = nc.declare_dram_parameter(
        "gwD", [128, 2 * KD * HF], bf16, isOutput=False
    )
    wtD = nc.declare_dram_parameter("wtD", [128, KD * 112], bf16, isOutput=False)
    # w2 includes the ragged tail as a 22nd chunk zero-padded to 128 rows,
    # so every y matmul keeps a full-row stationary (partial-row stationaries
    # break LDWEIGHTS pull-ahead and stall the PE ~200ns per chain).
    w2D = nc.declare_dram_parameter("w2D", [128, KH * D], bf16, isOutput=False)
    yD = nc.declare_dram_parameter("yD", [128, KD * C], f32, isOutput=True)

    with tile.TileContext(nc) as tc:
        with (
            tc.tile_pool(name="wpool", bufs=1) as wpool,
            tc.tile_pool(name="xpool", bufs=2) as xpool,
            tc.tile_pool(name="hpool", bufs=1) as hpool,
            tc.tile_pool(name="gpool", bufs=3) as gpool,
            tc.tile_pool(name="ypool", bufs=3) as ypool,
            tc.tile_pool(name="pg", bufs=2, space="PSUM") as pg,
            tc.tile_pool(name="pu", bufs=2, space="PSUM") as pu,
            tc.tile_pool(name="pt", bufs=1, space="PSUM") as pt,
            tc.tile_pool(name="py", bufs=3, space="PSUM") as py,
        ):
            tiles = _token_tiles(C)
            x_tiles = {}

            # PE warmup: the HAM clock gate holds the PE at half clock
            # until it sees ~3.4us of sustained activity. Run a chain of
            # throwaway matmuls on zeros (alternating pg/pu banks) while
            # the ramp DMAs are in flight, so the real stream starts at
            # full clock. The PE is otherwise idle here - this is free.
            ct_0 = tiles[0][1]
            warm = wpool.tile([128, ct_0], bf16, tag="warm")
            nc.vector.memset(warm[:], 0.0)
            # 10 matmuls bridge the gap from engine-ready (~8us) to first
            # real operands landing (~11.5us); once real matmuls flow the
            # PE stays busy and warms itself, so more would displace work.
            for i in range(10):
                if i % 2 == 0:
                    w_ps = pg.tile([128, ct_0], f32, tag="g_ps")
                else:
                    w_ps = pu.tile([128, ct_0], f32, tag="u_ps")
                nc.tensor.matmul(
                    w_ps[:], warm[:, 0:128], warm[:], start=True, stop=True
                )

            def load_x(t_idx, split=1):
                t0, ct = tiles[t_idx]
                x_s = xpool.tile([128, KD * ct], bf16, tag="x")
                w = KD * ct // split
                for c in range(split):
                    nc.sync.dma_start(
                        x_s[:, c * w:(c + 1) * w],
                        xD[:, KD * t0 + c * w:KD * t0 + (c + 1) * w],
                    )
                x_tiles[t_idx] = x_s

            # Ramp: tile 0's token slices load per-d, interleaved with the
            # finest wg/w1 group's quarters, so the first matmul chain
            # starts as early as possible (each dma_start trigger costs
            # ~650ns serially per engine). All input loads stay on the
            # Sync HWDGE queue: input loads on the Activation queue were
            # observed to race their consumers (intermittent corruption).
            x0 = xpool.tile([128, KD * ct_0], bf16, tag="x")
            x_tiles[0] = x0

            def load_x0_slice(d):
                nc.sync.dma_start(
                    x0[:, d * ct_0:(d + 1) * ct_0],
                    xD[:, d * ct_0:(d + 1) * ct_0],
                )

            n_grp = len(GRPS)
            gw_tiles = []
            for g in range(n_grp):
                cw = GRPS[g] * 128
                t = wpool.tile([128, 2 * KD * cw], bf16, tag=f"gw{g}")
                gw_tiles.append(t)
            cw0 = GRPS[0] * 128
            quart = KD * cw0 // 2
            for q in range(4):
                load_x0_slice(2 * q)
                load_x0_slice(2 * q + 1)
                nc.sync.dma_start(
                    gw_tiles[0][:, q * quart:(q + 1) * quart],
                    gwD[:, q * quart:(q + 1) * quart],
                )
            for g in range(1, n_grp):
                cw = GRPS[g] * 128
                base = 2 * KD * _GRP_OFF[g] * 128
                nc.sync.dma_start(
                    gw_tiles[g][:], gwD[:, base:base + 2 * KD * cw]
                )

            def wg_slice(d, hk):
                g, o = _HK_MAP[hk]
                cw = GRPS[g] * 128
                return gw_tiles[g][:, d * cw + o * 128:d * cw + (o + 1) * 128]

            def w1_slice(d, hk):
                g, o = _HK_MAP[hk]
                cw = GRPS[g] * 128
                base = KD * cw + d * cw + o * 128
                return gw_tiles[g][:, base:base + 128]

            # Merged ragged tail of wg/w1: [128, 112] per d-chunk.
            wt_s = wpool.tile([128, KD * 112], bf16, tag="wt")
            nc.sync.dma_start(wt_s[:], wtD[:])

            w2_tiles = []
            w2_off = [0]
            for s in W2GRPS:
                w2_off.append(w2_off[-1] + s)
            for g, s in enumerate(W2GRPS):
                t = wpool.tile([128, s * D], bf16, tag=f"w2{g}")
                nc.sync.dma_start(
                    t[:], w2D[:, w2_off[g] * D:w2_off[g + 1] * D]
                )
                w2_tiles.append(t)

            def w2_slice(hk, d):
                g = 0
                while hk >= w2_off[g + 1]:
                    g += 1
                base = (hk - w2_off[g]) * D + d * 128
                return w2_tiles[g][:, base:base + 128]

            for ti, (t0, ct) in enumerate(tiles):
                if ti + 1 < len(tiles):
                    load_x(ti + 1)
                x_s = x_tiles.pop(ti)

                h_s = hpool.tile([128, KH * ct], bf16, tag="h")
                for hk in range(KHF):
                    g_ps = pg.tile([128, ct], f32, tag="g_ps")
                    u_ps = pu.tile([128, ct], f32)
                    for d in range(KD):
                        nc.tensor.matmul(
                            g_ps[:],
                            wg_slice(d, hk),
                            x_s[:, d * ct:(d + 1) * ct],
                            start=(d == 0),
                            stop=(d == KD - 1),
                        )
                        nc.tensor.matmul(
                            u_ps[:],
                            w1_slice(d, hk),
                            x_s[:, d * ct:(d + 1) * ct],
                            start=(d == 0),
                            stop=(d == KD - 1),
                        )
                    g_tmp = gpool.tile([128, ct], f32, tag="g")
                    nc.scalar.activation(
                        g_tmp[:], g_ps[:], mybir.ActivationFunctionType.Silu
                    )
                    nc.vector.tensor_mul(
                        h_s[:, hk * ct:(hk + 1) * ct], g_tmp[:], u_ps[:]
                    )
                # ragged tail: one 8-matmul chain computes both g and u
                # (g rows on psum partitions 0:48, u rows on 64:112)
                t_ps = pt.tile([112, ct], f32, tag="t_ps")
                for d in range(KD):
                    nc.tensor.matmul(
                        t_ps[:],
                        wt_s[:, d * 112:(d + 1) * 112],
                        x_s[:, d * ct:(d + 1) * ct],
                        start=(d == 0),
                        stop=(d == KD - 1),
                    )
                gt_tmp = gpool.tile([TAIL, ct], f32, tag="gt")
                nc.scalar.activation(
                    gt_tmp[:], t_ps[0:TAIL], mybir.ActivationFunctionType.Silu
                )
                # rows TAIL:128 of the tail h chunk must be zeros (not
                # garbage) since the padded w2 tail chunk contracts over
                # all 128 partitions; memset the whole chunk first (offset
                # memsets are capped at 32 partitions) and let the mul
                # overwrite rows 0:TAIL.
                nc.vector.memset(h_s[:, KHF * ct:KH * ct], 0.0)
                nc.vector.tensor_mul(
                    h_s[0:TAIL, KHF * ct:KH * ct], gt_tmp[:], t_ps[64:64 + TAIL]
                )

                last = ti == len(tiles) - 1
                for d in range(KD):
                    if last and d == KD - 1:
                        # very last output chunk: compute/store in quarters
                        # so the copy+DMA after the final matmul is small
                        hw = ct // 4
                        for c0 in range(0, ct, hw):
                            y_ps = py.tile([128, hw], f32, tag="y_ps")
                            for hk in range(KH):
                                nc.tensor.matmul(
                                    y_ps[:],
                                    w2_slice(hk, d),
                                    h_s[:, hk * ct + c0:hk * ct + c0 + hw],
                                    start=(hk == 0),
                                    stop=(hk == KH - 1),
                                )
                            y_sb = ypool.tile([128, hw], f32, tag="yl")
                            nc.vector.tensor_copy(y_sb[:], y_ps[:])
                            nc.sync.dma_start(
                                yD[:, KD * t0 + d * ct + c0:
                                   KD * t0 + d * ct + c0 + hw],
                                y_sb[:],
                            )
                        continue
                    y_ps = py.tile([128, ct], f32)
                    for hk in range(KH):
                        nc.tensor.matmul(
                            y_ps[:],
                            w2_slice(hk, d),
                            h_s[:, hk * ct:(hk + 1) * ct],
                            start=(hk == 0),
                            stop=(hk == KH - 1),
                        )
                    # the last tile stores per-d so the final copy+DMA
                    # tail after the last matmul is as short as possible
                    if last:
                        y_sb = ypool.tile([128, ct], f32, tag="yl")
                        nc.vector.tensor_copy(y_sb[:], y_ps[:])
                        nc.sync.dma_start(
                            yD[:, KD * t0 + d * ct:KD * t0 + (d + 1) * ct],
                            y_sb[:],
                        )
                    elif d % 2 == 0:
                        y_pair = ypool.tile([128, 2 * ct], f32, tag="y")
                        nc.vector.tensor_copy(y_pair[:, 0:ct], y_ps[:])
                    else:
                        nc.vector.tensor_copy(y_pair[:, ct:2 * ct], y_ps[:])
                        nc.sync.dma_start(
                            yD[:, KD * t0 + (d - 1) * ct:KD * t0 + (d + 1) * ct],
                            y_pair[:],
                        )

    _PROGRAM_CACHE[C] = nc
    return nc


def _route(xf, gate_w):
    """Host router matching the reference: fp32 logits/softmax, top-2."""
    logits = xf @ gate_w.T  # [T, E] fp32
    m = logits.max(axis=1, keepdims=True)
    p = np.exp(logits - m, dtype=np.float32)
    p /= p.sum(axis=1, keepdims=True)
    # softmax is monotonic in logits, so top-2 by probs == top-2 by logits
    top_i = np.argsort(-p, axis=1, kind="stable")[:, :TOPK]  # [T, 2]
    top_p = np.take_along_axis(p, top_i, axis=1)
    gate_weights = top_p / (top_p.sum(axis=1, keepdims=True) + np.float32(1e-8))
    return top_i, gate_weights.astype(np.float32)


def _tileize(mat_T, splits):
    """[rows*128k, cols] -> [128, k*cols] blocks in device tile order.

    mat_T is [n*128, cols]; returns [128, n*cols] where block i holds
    rows i*128:(i+1)*128. `splits` optionally regroups the column axis
    first: a list of (c0, cw) column ranges, each emitted contiguously.
    """
    n = mat_T.shape[0] // 128
    a = mat_T.reshape(n, 128, mat_T.shape[1])
    blocks = []
    for c0, cw in splits:
        blocks.append(a[:, :, c0:c0 + cw].transpose(1, 0, 2).reshape(128, -1))
    return np.concatenate(blocks, axis=1) if len(blocks) > 1 else blocks[0]


def kernel(x, gate_w, Wg, W1, W2):
    from concourse.bass_utils import run_bass_kernel_spmd

    x = np.asarray(x, dtype=np.float32)
    gate_w = np.asarray(gate_w, dtype=np.float32)
    Wg = np.asarray(Wg, dtype=np.float32)
    W1 = np.asarray(W1, dtype=np.float32)
    W2 = np.asarray(W2, dtype=np.float32)

    xf = x.reshape(-1, D)
    top_i, gate_weights = _route(xf, gate_w)

    idx = [None] * E
    wts = [None] * E
    for e in range(E):
        rows, slots = np.nonzero(top_i == e)
        idx[e] = rows
        wts[e] = gate_weights[rows, slots]
    counts = np.array([len(i) for i in idx])
    C = min(CAP, max(128, int(np.ceil(counts.max() / 32)) * 32))
    dcounts = np.minimum(counts, C)

    nc = _build_program(C)
    tiles = _token_tiles(C)

    xf_bf = xf.astype(_BF16)
    in_maps = []
    for e in range(E):
        xT_e = np.zeros((D, C), dtype=_BF16)
        xT_e[:, : dcounts[e]] = xf_bf[idx[e][:C]].T
        wg_bf = Wg[e].astype(_BF16)  # [H, D]
        w1_bf = W1[e].astype(_BF16)
        # wg/w1 grouped blocks: per group g, [wg d0..d7 | w1 d0..d7]
        wgT = np.ascontiguousarray(wg_bf[:HF, :].T)  # [D, HF]
        w1T = np.ascontiguousarray(w1_bf[:HF, :].T)
        gw_blocks = []
        for g, s in enumerate(GRPS):
            c0, cw = _GRP_OFF[g] * 128, s * 128
            gw_blocks.append(_tileize(wgT, [(c0, cw)]))
            gw_blocks.append(_tileize(w1T, [(c0, cw)]))
        gwD_e = np.concatenate(gw_blocks, axis=1)
        # merged ragged tail [128, KD*112]
        wtT = np.zeros((D, 112), dtype=_BF16)
        wtT[:, 0:TAIL] = wg_bf[HF:H, :].T
        wtT[:, 64:64 + TAIL] = w1_bf[HF:H, :].T
        wtD_e = _tileize(wtT, [(0, 112)])
        w2T_pad = np.zeros((KH * 128, D), dtype=_BF16)
        w2T_pad[:H] = W2[e].astype(_BF16).T  # [H, D], tail rows zero
        in_maps.append({
            "xD": _tileize(xT_e, [(t0, ct) for t0, ct in tiles]),
            "gwD": gwD_e,
            "wtD": wtD_e,
            "w2D": _tileize(w2T_pad, [(0, D)]),
        })

    res = run_bass_kernel_spmd(nc, in_maps, list(range(N_CORES)))

    out = np.zeros((T, D), dtype=np.float32)
    for e in range(E):
        yD_e = res.results[e]["yD"]  # [128, KD*C] fp32, tile-major blocks
        yT_e = np.empty((C, D), dtype=np.float32)  # [token, D]
        for t0, ct in tiles:
            blk = yD_e[:, KD * t0:KD * (t0 + ct)].reshape(128, KD, ct)
            yT_e[t0:t0 + ct] = blk.transpose(2, 1, 0).reshape(ct, D)
        dev = idx[e][:C]
        out[dev] += wts[e][:C, None] * yT_e[: dcounts[e]]
        if counts[e] > C:  # host fp32 overflow path (capacity-limited MoE)
            ovf = idx[e][C:]
            xo = xf[ovf]
            go = xo @ Wg[e].T
            go = go / (1.0 + np.exp(-go))
            uo = xo @ W1[e].T
            yo = (go * uo) @ W2[e].T
            out[ovf] += wts[e][C:, None] * yo
    return out.reshape(B, S, D)
